# revision 7
# baseline (speedup 1.0000x reference)
"""Causal Grassmann Mixer — Trainium2 Bass kernel (8 NeuronCores, SPMD).

Sharding: data-parallel over B and sequence-parallel over L.
  core c -> batch b = c // 2, sequence half = c % 2 (2048 tokens each),
  plus a 32-token halo of h (the max offset) prepended on the host, so no
  cross-core communication is needed at all.

Device layout is feature-major everywhere: features on SBUF partitions,
tokens on the free dim.  The host pre-transposes h (and casts to bf16 +
fp8); the per-core output comes back feature-major and is transposed back
on the host.  All matmuls run in bf16 (fp32 PSUM accumulation) except the
gate, which runs entirely in fp8 DoubleRow (2 K-rows/cycle).

Math restructuring vs the reference:
  z = h@red_w (R=16) in bf16, then ZI/ZJ gathered to 120 plucker lanes by
     a one-hot matmul (exact), so the causal shift by d is just a column
     offset into the ZI/ZJ buffers.
  -> sum_d gelu(a_d) @ g2_w = (sum_d gelu(a_d)) @ g2_w : one g2 matmul.
  -> geom mean: count(t)=6 for t>=32; 1/6 is folded into g2_w on the host
     and the first 512 tokens of a sequence get an exact per-token
     correction vector (corr = 6/count, corr(0)=0) multiplied into S.
  -> gate: logits = h@W1 + g@W2 computed in ONE fp8 PSUM accumulation
     (DoubleRow, K=2048 total).  W1/W2 are scaled x64 on the host so the
     fp8 mantissa is fully used (they are ~N(0, 1/2048)); the sigmoid
     activation descales with scale=1/64.

Scheduling: weight loads are shared — every gate/g2 LDWEIGHTS feeds two
matmuls (both token tiles of a group, interleaved PSUM groups in separate
banks), the p1/p2 phases run at half-group granularity so the PE starts
earlier, and the elementwise load is spread over DVE, ACT and the
otherwise-idle Pool engine (blend diff h-g, S-accumulation tree).
"""

import numpy as np
import ml_dtypes

B, L, D = 4, 4096, 1024
R = 16
PLU = 120
DG = 256
OFFSETS = (1, 2, 4, 8, 16, 32)
HALO = 32
IDX_I, IDX_J = np.triu_indices(R, k=1)

NCORES = 8
TOK = 2048          # own tokens per core
TB = TOK + HALO     # token buffer incl. halo
T = 512             # token tile (one PSUM bank of fp32)
NT = TOK // T       # 4 output tiles per core
KD = D // 128       # 8 k-chunks of the model dim
WSC = 64.0          # fp8 gate-weight scale (descaled in the sigmoid)

BF16 = ml_dtypes.bfloat16

_CACHE = {}


def _build_program(gelu_name="Gelu"):
    import concourse.bass as bass
    import concourse.mybir as mybir
    import concourse.tile as tile
    from concourse import bacc

    f32 = mybir.dt.float32
    bf16 = mybir.dt.bfloat16
    f8 = mybir.dt.float8e4
    AF = mybir.ActivationFunctionType
    GELU = getattr(AF, gelu_name)
    DR = mybir.MatmulPerfMode.DoubleRow

    nc = bacc.Bacc(
        "TRN2",
        target_bir_lowering=False,
        debug=False,
        enable_asserts=False,
        num_devices=NCORES,
    )

    # ---- DRAM I/O ----
    h_t = nc.dram_tensor("h_t", [D, TB], bf16, kind="ExternalInput").ap()
    h8_t = nc.dram_tensor("h8_t", [D, TB], f8, kind="ExternalInput").ap()
    rw16 = nc.dram_tensor("rw16", [D, R], bf16, kind="ExternalInput").ap()
    rb16 = nc.dram_tensor("rb16", [R, 1], f32, kind="ExternalInput").ap()
    selij = nc.dram_tensor("selij", [R, 2 * PLU], bf16, kind="ExternalInput").ap()
    g1w = nc.dram_tensor("g1w", [PLU, DG], bf16, kind="ExternalInput").ap()
    g1b = nc.dram_tensor("g1b", [128, 2], f32, kind="ExternalInput").ap()
    g2w = nc.dram_tensor("g2w", [DG, D], bf16, kind="ExternalInput").ap()
    g2b = nc.dram_tensor("g2b", [128, KD], f32, kind="ExternalInput").ap()
    gw1 = nc.dram_tensor("gw1", [D, D], f8, kind="ExternalInput").ap()
    gw2 = nc.dram_tensor("gw2", [D, D], f8, kind="ExternalInput").ap()
    gtb = nc.dram_tensor("gtb", [128, KD], f32, kind="ExternalInput").ap()
    corr = nc.dram_tensor("corr", [1, T], bf16, kind="ExternalInput").ap()
    rsel_d = nc.dram_tensor("rsel", [12, 12 * PLU], bf16, kind="ExternalInput").ap()
    out_t = nc.dram_tensor("out_t", [D, TOK], bf16, kind="ExternalOutput").ap()

    with tile.TileContext(nc) as tc:
        from contextlib import ExitStack

        ctx = ExitStack()
        with ctx:
            singles = ctx.enter_context(tc.tile_pool(name="singles", bufs=1))
            work = ctx.enter_context(tc.tile_pool(name="work", bufs=3))
            acc = ctx.enter_context(tc.tile_pool(name="acc", bufs=2))
            psum = ctx.enter_context(tc.tile_pool(name="psum", bufs=4, space="PSUM"))
            psul = ctx.enter_context(tc.tile_pool(name="psul", bufs=4, space="PSUM"))

            # ---- resident SBUF tensors (weights first: small, unblock z) ----
            rw_sb = singles.tile([128, KD, R], bf16)
            nc.sync.dma_start(out=rw_sb, in_=rw16.rearrange("(c p) m -> p c m", p=128))
            rb_sb = singles.tile([R, 1], f32)
            nc.sync.dma_start(out=rb_sb, in_=rb16)
            sel_sb = singles.tile([R, 2 * PLU], bf16)
            nc.sync.dma_start(out=sel_sb, in_=selij)
            g1w_sb = singles.tile([PLU, DG], bf16)
            nc.sync.dma_start(out=g1w_sb, in_=g1w)
            g2w_sb = singles.tile([128, 2, D], bf16)
            nc.sync.dma_start(out=g2w_sb, in_=g2w.rearrange("(c p) m -> p c m", p=128))
            gw1_sb = singles.tile([128, KD, D], f8)
            nc.sync.dma_start(out=gw1_sb, in_=gw1.rearrange("(c p) m -> p c m", p=128))
            gw2_sb = singles.tile([128, KD, D], f8)
            nc.sync.dma_start(out=gw2_sb, in_=gw2.rearrange("(c p) m -> p c m", p=128))
            g1b_sb = singles.tile([128, 2], f32)
            nc.sync.dma_start(out=g1b_sb, in_=g1b)
            g2b_sb = singles.tile([128, KD], f32)
            nc.sync.dma_start(out=g2b_sb, in_=g2b)
            gtb_sb = singles.tile([128, KD], f32)
            nc.sync.dma_start(out=gtb_sb, in_=gtb)
            corr_sb = singles.tile([1, T], bf16)
            nc.sync.dma_start(out=corr_sb, in_=corr)

            ones_m = singles.tile([1, 128], bf16)
            nc.vector.memset(ones_m, 1.0)
            # one-hot columns: onehot[:, s, m] = (m == s): the 6 per-half
            # (offset) norm reductions accumulate onto 6 distinct PSUM rows
            onehot = singles.tile([PLU, 12, 12], bf16)
            nc.vector.memset(onehot, 0.0)
            for dcol in range(12):
                nc.vector.memset(onehot[:, dcol, dcol:dcol + 1], 1.0)
            magic = singles.tile([6, T], mybir.dt.int32)
            nc.vector.memset(magic, 0x5F375A86)  # Quake rsqrt seed
            # row selector+broadcast: rsel[k, d, m] = (k == d); lhsT for the
            # matmul that broadcasts rinv row d across 120 partitions
            rsel = singles.tile([12, 12, PLU], bf16)
            nc.sync.dma_start(out=rsel, in_=rsel_d.rearrange("k (d m) -> k d m", m=PLU))

            # h in bf16 (z-phase rhs + blend), chunk-major so the z phase
            # can start as soon as the first token-chunk arrives
            h_sb = singles.tile([128, KD, TB], bf16)
            h_r = h_t.rearrange("(c p) t -> p c t", p=128)
            zchunks = [(c * T, min(T, TB - c * T)) for c in range((TB + T - 1) // T)]
            for (c0, csz) in zchunks:
                for k in range(KD):
                    nc.sync.dma_start(
                        out=h_sb[:, k, c0:c0 + csz], in_=h_r[:, k, c0:c0 + csz]
                    )
            # h in fp8 (gate rhs) — only needed once the gate starts
            h8_sb = singles.tile([128, KD, TB], f8)
            h8_r = h8_t.rearrange("(c p) t -> p c t", p=128)
            for k in range(KD):
                nc.sync.dma_start(out=h8_sb[:, k, :], in_=h8_r[:, k, :])

            zi_sb = singles.tile([PLU, TB], bf16)
            zj_sb = singles.tile([PLU, TB], bf16)
            pp_pool = ctx.enter_context(tc.tile_pool(name="pp", bufs=1))
            s_pool = ctx.enter_context(tc.tile_pool(name="spool", bufs=1))
            gfm_pool = ctx.enter_context(tc.tile_pool(name="gfmpool", bufs=1))
            sq_pool = ctx.enter_context(tc.tile_pool(name="sqp", bufs=1))
            st = {}

            # ---- phase Z: z = h@red_w + red_b (R=16), then one-hot gather
            # to ZI/ZJ (exact pass-through).  k-outer so each red_w /
            # selector weight tile is loaded once per sweep. ----
            def zphase(chunks):
                zps = {}
                for (c0, csz) in chunks:
                    zps[c0] = psum.tile([R, csz], f32, tag="ps", name=f"zp{c0}")
                for k in range(KD):
                    for (c0, csz) in chunks:
                        nc.tensor.matmul(
                            zps[c0],
                            lhsT=rw_sb[:, k, :],
                            rhs=h_sb[:, k, c0:c0 + csz],
                            start=(k == 0),
                            stop=(k == KD - 1),
                        )
                z16s = {}
                for (c0, csz) in chunks:
                    z16 = work.tile([R, csz], bf16, tag="z16", bufs=3)
                    nc.vector.tensor_scalar_add(z16, zps[c0], rb_sb)
                    z16s[c0] = z16
                for g, z_sb in ((0, zi_sb), (1, zj_sb)):
                    gps = {}
                    for (c0, csz) in chunks:
                        gp = psum.tile([PLU, csz], f32, tag="ps", name=f"gp{g}_{c0}")
                        nc.tensor.matmul(
                            gp,
                            lhsT=sel_sb[:, g * PLU:(g + 1) * PLU],
                            rhs=z16s[c0],
                            start=True,
                            stop=True,
                        )
                        gps[c0] = gp
                    for (c0, csz) in chunks:
                        nc.vector.tensor_copy(out=z_sb[:, c0:c0 + csz], in_=gps[c0])

            out_r = out_t.rearrange("(c p) t -> p c t", p=128)
            GT = 2 * T  # two tiles per phase group
            NG = NT // 2

            def p1a(grp, i):
                """DVE-only: plucker p and p^2 for half-tile i of the group."""
                g0 = HALO + (2 * grp + i) * T
                if i == 0:
                    pp = pp_pool.tile([PLU, 6, GT], bf16, name=f"pp{grp}", tag="pp")
                    sq6 = sq_pool.tile([PLU, 6, GT], bf16, name=f"sq{grp}", tag="sq")
                    st[grp] = {"pp": pp, "sq6": sq6}
                pp, sq6 = st[grp]["pp"], st[grp]["sq6"]
                sl = slice(i * T, (i + 1) * T)
                for di, delta in enumerate(OFFSETS):
                    past = slice(g0 - delta, g0 - delta + T)
                    cur = slice(g0, g0 + T)
                    m1 = work.tile([PLU, T], bf16)
                    nc.vector.tensor_mul(m1, zi_sb[:, past], zj_sb[:, cur])
                    m2 = work.tile([PLU, T], bf16)
                    nc.vector.tensor_mul(m2, zj_sb[:, past], zi_sb[:, cur])
                    nc.vector.tensor_sub(pp[:, di, sl], m1, m2)
                    nc.vector.tensor_mul(sq6[:, di, sl], pp[:, di, sl], pp[:, di, sl])

            def p1b(grp, i):
                """Norm reduce (PE), one batched rsqrt (DVE), broadcast+scale."""
                pp, sq6 = st[grp]["pp"], st[grp]["sq6"]
                ns6 = psum.tile([6, T], f32, tag="ps", name=f"ns6_{grp}_{i}")
                for di in range(6):
                    nc.tensor.matmul(
                        ns6,
                        lhsT=onehot[:, di, :6],
                        rhs=sq6[:, di, i * T:(i + 1) * T],
                        start=(di == 0),
                        stop=(di == 5),
                    )
                # rinv = rsqrt(ns + EPS^2): Quake seed + 1 Newton step
                nsf = work.tile([6, T], f32, tag="rs", bufs=4)
                nc.vector.tensor_scalar_add(nsf, ns6, 1e-12)
                sh = work.tile([6, T], mybir.dt.int32, tag="rs", bufs=4)
                nc.vector.tensor_scalar(
                    sh, nsf.bitcast(mybir.dt.int32), 1, None,
                    op0=mybir.AluOpType.arith_shift_right,
                )
                y0 = work.tile([6, T], f32, tag="rs", bufs=4)
                nc.vector.tensor_sub(y0.bitcast(mybir.dt.int32), magic, sh)
                t1 = work.tile([6, T], f32, tag="rs", bufs=4)
                nc.vector.tensor_mul(t1, y0, y0)
                nc.vector.tensor_mul(t1, t1, nsf)
                nc.vector.tensor_scalar(
                    t1, t1, -0.5, 1.5,
                    op0=mybir.AluOpType.mult, op1=mybir.AluOpType.add,
                )
                rinv = work.tile([6, T], bf16)
                nc.vector.tensor_mul(rinv, y0, t1)
                sl = slice(i * T, (i + 1) * T)
                for di in range(6):
                    rb = psum.tile([PLU, T], f32, tag="ps")
                    nc.tensor.matmul(
                        rb, lhsT=rsel[:6, di, :], rhs=rinv,
                        start=True, stop=True,
                    )
                    nc.vector.tensor_mul(pp[:, di, sl], pp[:, di, sl], rb)

            def p2part(grp, i):
                """a_d = p@g1_w + g1_b; S = sum_d gelu(a_d).
                Pool pair-adds the middle of the 6-way sum; DVE finishes."""
                pp = st[grp]["pp"]
                if i == 0:
                    st[grp]["s"] = s_pool.tile(
                        [128, 2, 2, T], bf16, name=f"s{grp}", tag="s")
                s_sb = st[grp]["s"]
                for m in range(2):
                    gt = {}
                    for di in range(6):
                        ap_ps = psum.tile([128, T], f32, tag="ps")
                        nc.tensor.matmul(
                            ap_ps,
                            lhsT=g1w_sb[:, m * 128:(m + 1) * 128],
                            rhs=pp[:, di, i * T:(i + 1) * T],
                            start=True,
                            stop=True,
                        )
                        if di == 0:
                            nc.scalar.activation(
                                s_sb[:, m, i, :], ap_ps, GELU,
                                bias=g1b_sb[:, m:m + 1],
                            )
                        else:
                            g = acc.tile([128, T], bf16, tag=f"g{di}")
                            nc.scalar.activation(
                                g, ap_ps, GELU, bias=g1b_sb[:, m:m + 1]
                            )
                            gt[di] = g
                    t1 = acc.tile([128, T], bf16, tag="t1")
                    nc.vector.tensor_add(t1, gt[1], gt[2])
                    t2 = acc.tile([128, T], bf16, tag="t2")
                    nc.vector.tensor_add(t2, gt[3], gt[4])
                    t3 = acc.tile([128, T], bf16, tag="t3")
                    nc.vector.tensor_add(t3, t1, t2)
                    nc.vector.tensor_add(s_sb[:, m, i, :], s_sb[:, m, i, :], gt[5])
                    nc.vector.tensor_add(s_sb[:, m, i, :], s_sb[:, m, i, :], t3)
                if grp == 0 and i == 0:
                    # first-tile count correction (corr==1 for t>=32)
                    corr_ps = psum.tile([128, T], f32, tag="ps")
                    nc.tensor.matmul(
                        corr_ps, lhsT=ones_m, rhs=corr_sb, start=True, stop=True
                    )
                    for m in range(2):
                        nc.vector.tensor_mul(
                            s_sb[:, m, 0, :], s_sb[:, m, 0, :], corr_ps
                        )

            def gpart(grp):
                """G = S @ (g2_w/6) + g2_b, bf16 (blend) + fp8 (gate rhs);
                each g2 weight tile is loaded once for both token tiles.
                Pool precomputes dd = h - G for the blend."""
                s_sb = st[grp]["s"]
                gfm_sb = gfm_pool.tile(
                    [128, KD, 2, T], bf16, name=f"gfm{grp}", tag="gfm")
                gfm8_sb = gfm_pool.tile(
                    [128, KD, 2, T], f8, name=f"gfm8{grp}", tag="gfm8")
                dd_sb = gfm_pool.tile(
                    [128, KD, 2, T], bf16, name=f"dd{grp}", tag="dd")
                st[grp]["gfm"] = gfm_sb
                st[grp]["gfm8"] = gfm8_sb
                st[grp]["dd"] = dd_sb
                g0 = HALO + 2 * grp * T
                for m8 in range(KD):
                    gp = [psum.tile([128, T], f32, tag="ps", name=f"gp{m8}_{i}")
                          for i in range(2)]
                    for k2 in range(2):
                        w = g2w_sb[:, k2, m8 * 128:(m8 + 1) * 128]
                        for i in range(2):
                            nc.tensor.matmul(
                                gp[i],
                                lhsT=w,
                                rhs=s_sb[:, k2, i, :],
                                start=(k2 == 0),
                                stop=(k2 == 1),
                            )
                    for i in range(2):
                        # bf16 copy for the blend: ACT for i=0, DVE for i=1
                        if i == 0:
                            nc.scalar.add(
                                gfm_sb[:, m8, i, :], gp[i], g2b_sb[:, m8:m8 + 1]
                            )
                        else:
                            nc.vector.tensor_scalar_add(
                                gfm_sb[:, m8, i, :], gp[i], g2b_sb[:, m8:m8 + 1]
                            )
                        nc.scalar.add(
                            gfm8_sb[:, m8, i, :], gp[i], g2b_sb[:, m8:m8 + 1]
                        )
                        cur = slice(g0 + i * T, g0 + (i + 1) * T)
                        nc.vector.tensor_sub(
                            dd_sb[:, m8, i, :], h_sb[:, m8, cur], gfm_sb[:, m8, i, :]
                        )

            def bphase(grp, m8s):
                """gate logits: ONE fp8 DoubleRow accumulation over h8 (K=1024)
                and gfm8 (K=1024) for BOTH tiles of the group — every weight
                tile is loaded once and used by two matmuls (interleaved PSUM
                groups in separate banks).  Sigmoid descales the x64 weights;
                blend is 2 DVE ops (dd precomputed on Pool)."""
                gfm_sb = st[grp]["gfm"]
                gfm8_sb = st[grp]["gfm8"]
                dd_sb = st[grp]["dd"]
                g0 = HALO + 2 * grp * T
                for m8 in m8s:
                    lp = [psul.tile([128, T], f32, tag="lp", name=f"lp{m8}_{i}")
                          for i in range(2)]
                    ms = slice(m8 * 128, (m8 + 1) * 128)
                    for kp in range(KD // 2):
                        w = gw1_sb[:, 2 * kp:2 * kp + 2, ms]
                        for i in range(2):
                            cur = slice(g0 + i * T, g0 + (i + 1) * T)
                            nc.tensor.matmul(
                                lp[i],
                                lhsT=w,
                                rhs=h8_sb[:, 2 * kp:2 * kp + 2, cur],
                                start=(kp == 0),
                                stop=False,
                                perf_mode=DR,
                            )
                    for kp in range(KD // 2):
                        w = gw2_sb[:, 2 * kp:2 * kp + 2, ms]
                        for i in range(2):
                            nc.tensor.matmul(
                                lp[i],
                                lhsT=w,
                                rhs=gfm8_sb[:, 2 * kp:2 * kp + 2, i, :],
                                start=False,
                                stop=(kp == KD // 2 - 1),
                                perf_mode=DR,
                            )
                    for i in range(2):
                        ti = 2 * grp + i
                        alpha = work.tile([128, T], bf16)
                        nc.scalar.activation(
                            alpha, lp[i], AF.Sigmoid, bias=gtb_sb[:, m8:m8 + 1],
                            scale=1.0 / WSC,
                        )
                        mm = work.tile([128, T], bf16)
                        nc.vector.tensor_mul(mm, alpha, dd_sb[:, m8, i, :])
                        oo = work.tile([128, T], bf16)
                        nc.vector.tensor_add(oo, gfm_sb[:, m8, i, :], mm)
                        nc.sync.dma_start(
                            out=out_r[:, m8, ti * T:(ti + 1) * T], in_=oo
                        )

            # software pipeline: the DVE-heavy plucker of group g+1 runs
            # while the PE crunches the fp8 gate of group g; the p1b/p2/g2
            # PE work of g+1 fills the space between the two gate halves.
            zphase(zchunks[:3])
            p1a(0, 0)
            zphase(zchunks[3:])
            p1b(0, 0); p2part(0, 0)
            p1a(0, 1)
            p1b(0, 1); p2part(0, 1)
            gpart(0)
            for grp in range(NG - 1):
                bphase(grp, range(0, KD // 2))
                p1a(grp + 1, 0)
                p1a(grp + 1, 1)
                bphase(grp, range(KD // 2, KD))
                p1b(grp + 1, 0); p2part(grp + 1, 0)
                p1b(grp + 1, 1); p2part(grp + 1, 1)
                gpart(grp + 1)
            bphase(NG - 1, range(0, KD // 2))
            bphase(NG - 1, range(KD // 2, KD))

    nc.compile()
    return nc


def _get_program():
    if "nc" not in _CACHE:
        _CACHE["nc"] = _build_program()
    return _CACHE["nc"]


def make_in_maps(h, red_w, red_b, g1_w, g1_b, g2_w, g2_b, gate_w, gate_b):
    """Host-side sharding + layout prep. Returns list of 8 input dicts."""
    h = np.asarray(h, np.float32)
    red_w = np.asarray(red_w, np.float32)
    red_b = np.asarray(red_b, np.float32)
    g1_w = np.asarray(g1_w, np.float32)
    g1_b = np.asarray(g1_b, np.float32)
    g2_w = np.asarray(g2_w, np.float32)
    g2_b = np.asarray(g2_b, np.float32)
    gate_w = np.asarray(gate_w, np.float32)
    gate_b = np.asarray(gate_b, np.float32)

    from concourse import mybir as _mb
    F8 = _mb.dt.np(_mb.dt.float8e4)

    rw16 = np.ascontiguousarray(red_w.astype(BF16))
    rb16 = np.ascontiguousarray(red_b.reshape(R, 1))
    # gather matrix: selij[r, k] = (r == IDX[k]) for ZI then ZJ
    selij = np.zeros((R, 2 * PLU), np.float32)
    for k in range(PLU):
        selij[IDX_I[k], k] = 1.0
        selij[IDX_J[k], PLU + k] = 1.0
    selij = np.ascontiguousarray(selij.astype(BF16))
    g1w = np.ascontiguousarray(g1_w.astype(BF16))
    g1b = np.ascontiguousarray(g1_b.reshape(2, 128).T.astype(np.float32))
    g2w = np.ascontiguousarray((g2_w / 6.0).astype(BF16))
    g2b = np.ascontiguousarray(g2_b.reshape(KD, 128).T.astype(np.float32))
    gw1 = np.ascontiguousarray((gate_w[:D] * WSC).astype(F8))
    gw2 = np.ascontiguousarray((gate_w[D:] * WSC).astype(F8))
    gtb = np.ascontiguousarray(gate_b.reshape(KD, 128).T.astype(np.float32))

    # per-token count correction for the first tile of a sequence
    t = np.arange(T)
    count = np.zeros(T, np.float32)
    for d in OFFSETS:
        count += (t >= d)
    corr0 = np.where(count > 0, 6.0 / np.maximum(count, 1.0), 0.0).astype(BF16)
    corr0 = corr0.reshape(1, T)
    corr1 = np.ones((1, T), BF16)

    rsel = np.zeros((12, 12, PLU), np.float32)
    for dd in range(12):
        rsel[dd, dd, :] = 1.0
    rsel = np.ascontiguousarray(rsel.reshape(12, 12 * PLU).astype(BF16))

    in_maps = []
    for c in range(NCORES):
        b, half = c // 2, c % 2
        if half == 0:
            pad = np.zeros((HALO, D), np.float32)
        else:
            pad = h[b, half * TOK - HALO: half * TOK]
        hs = np.concatenate([pad, h[b, half * TOK:(half + 1) * TOK]], axis=0)
        h_t = np.ascontiguousarray(hs.T.astype(BF16))  # (D, TB)
        h8_t = np.ascontiguousarray(hs.T.astype(F8))
        in_maps.append({
            "h_t": h_t,
            "h8_t": h8_t,
            "rw16": rw16,
            "rb16": rb16,
            "selij": selij,
            "g1w": g1w,
            "g1b": g1b,
            "g2w": g2w,
            "g2b": g2b,
            "gw1": gw1,
            "gw2": gw2,
            "gtb": gtb,
            "corr": corr0 if half == 0 else corr1,
            "rsel": rsel,
        })
    return in_maps


def assemble_output(results):
    out = np.empty((B, L, D), np.float32)
    for c in range(NCORES):
        b, half = c // 2, c % 2
        ot = np.asarray(results[c]["out_t"]).astype(np.float32)  # (D, TOK)
        out[b, half * TOK:(half + 1) * TOK, :] = ot.T
    return out


def kernel(**inputs):
    from concourse.bass_utils import run_bass_kernel_spmd

    nc = _get_program()
    in_maps = make_in_maps(**inputs)
    res = run_bass_kernel_spmd(nc, in_maps, core_ids=list(range(NCORES)))
    return assemble_output(res.results)
